# revision 6
# baseline (speedup 1.0000x reference)
"""MoE 2D router kernel for 8 Trainium2 NeuronCores.

Strategy (pure data parallel, batch-sharded):
  - B=16 batches split across 8 cores (2 per core). Per core, each batch's
    [C=16, H=128, W=128] tensor is viewed as [128, 2048] in SBUF with
    partition p = c*8 + blk (blk = pixel-block of 2048 contiguous pixels),
    so channel params are per-partition scalars and HBM loads are fully
    contiguous.
  - Expert-axis (C) reductions (top-2 max, softmax sum) are done by
    PE-transposing Hlogits chunks to pixel-major layout (PE f32 transpose is
    bit-exact), then free-axis strided tensor_reduce. The argmax mask is an
    exact is_equal in transposed space (step-0 broadcast APs), transposed
    back with bf16 (0/1 values are exact in bf16).
  - Per-pixel m1/m2 values are broadcast back to (c, pixel) layout with
    0/1 selection matmuls on the PE (bit-exact for x*1.0 accumulation).
  - softplus(t) = Ln(1 + Exp(t)) (no softplus table on gen3);
    erf(q) = 2*(DGelu(sqrt2*q) - 1.12838*q*exp(-q^2)) - 1 (no erf table);
    softmax is computed without max subtraction (|Hl| < 30, checked).
  - Work is split across DVE (vector), Pool (gpsimd) and ACT (scalar)
    engines to stay near the HBM roofline.
"""
import sys

sys.path.insert(0, "/opt/trn_rl_repo")

import numpy as np

B, C, H, W = 16, 16, 128, 128
NCORES = 8
BPC = B // NCORES           # batches per core
HW = H * W                  # 16384 pixels per (batch, channel)
NBLK = 8                    # pixel blocks per batch (HW / 2048)
FB = C * HW // 128          # free size per batch in [128, FB] layout = 2048
NCH = 4                     # chunks per batch
CHW = FB // NCH             # chunk width = 512
NG = FB // 128              # 128-col groups per batch = 16

_CACHE = {}


def _build():
    import concourse.bacc as bacc
    import concourse.mybir as mybir
    from concourse.tile import TileContext

    f32 = mybir.dt.float32
    bf16 = mybir.dt.bfloat16
    AX = mybir.AxisListType
    OP = mybir.AluOpType
    AF = mybir.ActivationFunctionType
    SQRT2 = 1.4142135623730951
    C_ERF = 1.1283791670955126  # 2/sqrt(pi)
    BIGNEG = -1e30

    nc = bacc.Bacc(trn_type="TRN2", target_bir_lowering=False, debug=False,
                   num_devices=NCORES, name="moe_router")

    xd = nc.dram_tensor("x", [BPC, 128, FB], f32, kind="ExternalInput")
    nd = nc.dram_tensor("noise", [BPC, 128, FB], f32, kind="ExternalInput")
    wgp_d = nc.dram_tensor("wgp", [128, 1], f32, kind="ExternalInput")
    wnp_d = nc.dram_tensor("wnp", [128, 1], f32, kind="ExternalInput")
    id_f = nc.dram_tensor("id_f", [128, 128], f32, kind="ExternalInput")
    id_b = nc.dram_tensor("id_b", [128, 128], bf16, kind="ExternalInput")
    sel64_d = nc.dram_tensor("sel64", [128, 1024], f32, kind="ExternalInput")
    selsum_d = nc.dram_tensor("selsum", [128, 128], f32, kind="ExternalInput")
    gd = nc.dram_tensor("g_out", [BPC, 128, FB], f32, kind="ExternalOutput")
    ld = nc.dram_tensor("load_out", [BPC, 128, FB], f32, kind="ExternalOutput")

    with TileContext(nc) as tc:
        with tc.tile_pool(name="const", bufs=1) as cpool, \
             tc.tile_pool(name="io", bufs=2) as iop, \
             tc.tile_pool(name="work", bufs=1) as wp, \
             tc.tile_pool(name="chunk", bufs=3) as chp, \
             tc.tile_pool(name="ps_t", bufs=2, space="PSUM") as ps_t, \
             tc.tile_pool(name="ps_m", bufs=1, space="PSUM") as ps_m, \
             tc.tile_pool(name="ps_s", bufs=1, space="PSUM") as ps_s, \
             tc.tile_pool(name="ps_b", bufs=1, space="PSUM") as ps_b:

            wgp = cpool.tile([128, 1], f32, tag="wgp")
            nc.sync.dma_start(out=wgp[:, :], in_=wgp_d[:, :])
            wnp = cpool.tile([128, 1], f32, tag="wnp")
            nc.sync.dma_start(out=wnp[:, :], in_=wnp_d[:, :])
            idf = cpool.tile([128, 128], f32, tag="idf")
            nc.sync.dma_start(out=idf[:, :], in_=id_f[:, :])
            idb = cpool.tile([128, 128], bf16, tag="idb")
            nc.sync.dma_start(out=idb[:, :], in_=id_b[:, :])
            sel64 = cpool.tile([128, 1024], f32, tag="sel64")
            nc.sync.dma_start(out=sel64[:, :], in_=sel64_d[:, :])
            selsum = cpool.tile([128, 128], f32, tag="selsum")
            nc.sync.dma_start(out=selsum[:, :], in_=selsum_d[:, :])

            for b in range(BPC):
                # ---- load ----
                xt = iop.tile([128, FB], f32, tag="x")
                nc.sync.dma_start(out=xt[:, :], in_=xd[b, :, :])
                nt = iop.tile([128, FB], f32, tag="noise")
                nc.sync.dma_start(out=nt[:, :], in_=nd[b, :, :])

                # ---- gates (A-space) ----
                # softplus(t) = Ln(1 + e^t); e^t Newton-refined via the Ln
                # table (y' = y*(1 + t - ln(y))) to kill the Exp-table error.
                tv = wp.tile([128, FB], f32, tag="tv")
                nc.vector.tensor_scalar_mul(tv[:, :], xt[:, :], wnp[:, :])
                eu0 = wp.tile([128, FB], f32, tag="eu")
                nc.scalar.activation(eu0[:, :], xt[:, :], AF.Exp, scale=wnp[:, :])
                lc = wp.tile([128, FB], f32, tag="lc")
                nc.scalar.activation(lc[:, :], eu0[:, :], AF.Ln)
                d2 = wp.tile([128, FB], f32, tag="d2")
                nc.gpsimd.tensor_tensor(d2[:, :], tv[:, :], lc[:, :], op=OP.subtract)
                eu = wp.tile([128, FB], f32, tag="eu2")
                nc.vector.scalar_tensor_tensor(eu[:, :], d2[:, :], 1.0, eu0[:, :],
                                               op0=OP.add, op1=OP.mult)
                wnoise = wp.tile([128, FB], f32, tag="wnoise")
                nc.scalar.activation(wnoise[:, :], eu[:, :], AF.Ln, bias=1.0)
                nw = wp.tile([128, FB], f32, tag="nw")
                nc.gpsimd.tensor_tensor(nw[:, :], nt[:, :], wnoise[:, :], op=OP.mult)
                wg = wp.tile([128, FB], f32, tag="wg")
                nc.vector.tensor_scalar_mul(wg[:, :], xt[:, :], wgp[:, :])
                hl = wp.tile([128, FB], f32, tag="hl")
                nc.vector.tensor_tensor(hl[:, :], wg[:, :], nw[:, :], op=OP.add)
                et = wp.tile([128, FB], f32, tag="e")
                nc.scalar.activation(et[:, :], hl[:, :], AF.Exp)

                # ---- per-pixel reductions via transposed chunks ----
                m1pm = wp.tile([128, 128], f32, tag="m1pm")
                m2pm = wp.tile([128, 128], f32, tag="m2pm")
                mask_sb = wp.tile([128, FB], bf16, tag="mask")
                srecip = wp.tile([128, FB], f32, tag="srecip")
                for ch in range(NCH):
                    cs = ch * CHW
                    hlT = ps_t.tile([128, CHW], f32, tag="hlT")
                    for g in range(NCH):
                        nc.tensor.transpose(
                            hlT[:, g * 128:(g + 1) * 128],
                            hl[:, cs + g * 128:cs + (g + 1) * 128], idf[:, :])
                    vT = hlT[:, :].rearrange("p (g c k) -> p g k c", g=NCH, c=C)
                    nc.vector.tensor_reduce(
                        m1pm[:, ch * 32:(ch + 1) * 32], vT, axis=AX.X, op=OP.max)
                    m1b = (m1pm[:, ch * 32:(ch + 1) * 32]
                           .rearrange("p (g k) -> p g k", g=NCH)
                           .unsqueeze(2).broadcast_to([128, NCH, C, NBLK]))
                    maskT = chp.tile([128, CHW], bf16, tag="maskT")
                    nc.vector.tensor_tensor(maskT[:, :], hlT[:, :], m1b, op=OP.is_equal)
                    maskd = chp.tile([128, CHW], f32, tag="maskd")
                    nc.vector.scalar_tensor_tensor(
                        maskd[:, :], maskT[:, :], BIGNEG, hlT[:, :],
                        op0=OP.mult, op1=OP.add)
                    vM = maskd[:, :].rearrange("p (g c k) -> p g k c", g=NCH, c=C)
                    nc.vector.tensor_reduce(
                        m2pm[:, ch * 32:(ch + 1) * 32], vM, axis=AX.X, op=OP.max)
                    # mask back to A-space (bf16 transpose is exact for 0/1)
                    maskA = ps_m.tile([128, CHW], bf16, tag="maskA")
                    for g in range(NCH):
                        nc.tensor.transpose(
                            maskA[:, g * 128:(g + 1) * 128],
                            maskT[:, g * 128:(g + 1) * 128], idb[:, :])
                    nc.vector.tensor_copy(mask_sb[:, cs:cs + CHW], maskA[:, :])
                    # softmax denominator (+ broadcast over c) on PE
                    ssum = ps_s.tile([128, CHW], f32, tag="ssum")
                    nc.tensor.matmul(ssum[:, :], selsum[:, :], et[:, cs:cs + CHW])
                    nc.vector.reciprocal(srecip[:, cs:cs + CHW], ssum[:, :])

                # ---- m1/m2' to row-major for PE broadcast ----
                m2p = wp.tile([128, 128], f32, tag="m2p")
                nc.vector.tensor_tensor(m2p[:, :], m2pm[:, :], m1pm[:, :], op=OP.subtract)
                m1Tp = ps_b.tile([128, 128], f32, tag="m1Tp")
                nc.tensor.transpose(m1Tp[:, :], m1pm[:, :], idf[:, :])
                m1T = wp.tile([128, 128], f32, tag="m1T")
                nc.vector.tensor_copy(m1T[:, :], m1Tp[:, :])
                m2Tp = ps_b.tile([128, 128], f32, tag="m1Tp")
                nc.tensor.transpose(m2Tp[:, :], m2p[:, :], idf[:, :])
                m2T = wp.tile([128, 128], f32, tag="m2T")
                nc.vector.tensor_copy(m2T[:, :], m2Tp[:, :])

                # ---- load-loss numerator (A-space) ----
                n1 = wp.tile([128, FB], f32, tag="n1")
                mm = wp.tile([128, FB], f32, tag="mm")
                for ch in range(NCH):
                    cs = ch * CHW
                    m1bA = ps_b.tile([128, CHW], f32, tag="m1bA")
                    m2bA = ps_b.tile([128, CHW], f32, tag="m2bA")
                    for g in range(NCH):
                        gg = ch * NCH + g
                        a64, j = divmod(gg, 8)
                        lhs = sel64[64 * a64:64 * (a64 + 1), j * 128:(j + 1) * 128]
                        nc.tensor.matmul(m1bA[:, g * 128:(g + 1) * 128],
                                         lhs, m1T[64 * a64:64 * (a64 + 1), :])
                        nc.tensor.matmul(m2bA[:, g * 128:(g + 1) * 128],
                                         lhs, m2T[64 * a64:64 * (a64 + 1), :])
                    nc.vector.tensor_tensor(n1[:, cs:cs + CHW], wg[:, cs:cs + CHW],
                                            m1bA[:, :], op=OP.subtract)
                    nc.vector.tensor_tensor(mm[:, cs:cs + CHW],
                                            mask_sb[:, cs:cs + CHW],
                                            m2bA[:, :], op=OP.mult)

                # ---- outputs ----
                g0 = wp.tile([128, FB], f32, tag="g0")
                nc.vector.tensor_tensor(g0[:, :], mask_sb[:, :], srecip[:, :], op=OP.mult)
                gt = iop.tile([128, FB], f32, tag="g")
                nc.gpsimd.tensor_tensor(gt[:, :], g0[:, :], et[:, :], op=OP.mult)
                nc.sync.dma_start(out=gd[b, :, :], in_=gt[:, :])

                rw = wp.tile([128, FB], f32, tag="rw")
                nc.vector.reciprocal(rw[:, :], wnoise[:, :])
                numer = wp.tile([128, FB], f32, tag="nw")
                nc.gpsimd.tensor_tensor(numer[:, :], n1[:, :], mm[:, :], op=OP.subtract)
                qt = wp.tile([128, FB], f32, tag="hl")
                nc.gpsimd.tensor_tensor(qt[:, :], numer[:, :], rw[:, :], op=OP.mult)
                # erf(q) = 2*(DGelu(sqrt2 q) - 1.12838*q*exp(-q^2)) - 1
                z2 = wp.tile([128, FB], f32, tag="eu")
                nc.gpsimd.tensor_tensor(z2[:, :], qt[:, :], qt[:, :], op=OP.mult)
                wt = wp.tile([128, FB], f32, tag="wg")
                nc.scalar.activation(wt[:, :], z2[:, :], AF.Exp, scale=-1.0)
                dg = wp.tile([128, FB], f32, tag="e")
                nc.scalar.activation(dg[:, :], qt[:, :], AF.Derivative_Gelu, scale=SQRT2)
                t2 = wp.tile([128, FB], f32, tag="n1")
                nc.vector.scalar_tensor_tensor(t2[:, :], qt[:, :], C_ERF, wt[:, :],
                                               op0=OP.mult, op1=OP.mult)
                er = wp.tile([128, FB], f32, tag="mm")
                nc.vector.scalar_tensor_tensor(er[:, :], dg[:, :], 2.0, t2[:, :],
                                               op0=OP.mult, op1=OP.subtract)
                lt = iop.tile([128, FB], f32, tag="load")
                nc.vector.tensor_scalar(lt[:, :], er[:, :], 1.0, None, op0=OP.subtract)
                nc.sync.dma_start(out=ld[b, :, :], in_=lt[:, :])

    nc.compile()
    return nc


def _consts():
    import ml_dtypes
    identity = np.eye(128, dtype=np.float32)
    # sel64[64a + j*8 + blk, j*128 + c*8 + blk] = 1  -> out[(c,blk), col_j] = rhs[(j,blk)-row]
    sel64 = np.zeros((128, 1024), dtype=np.float32)
    for a in range(2):
        for j in range(8):
            for blk in range(8):
                for c in range(C):
                    sel64[64 * a + j * 8 + blk, j * 128 + c * 8 + blk] = 1.0
    selsum = np.zeros((128, 128), dtype=np.float32)
    for cp in range(C):
        for blk in range(8):
            for c in range(C):
                selsum[cp * 8 + blk, c * 8 + blk] = 1.0
    return {
        "id_f": identity,
        "id_b": identity.astype(ml_dtypes.bfloat16),
        "sel64": sel64,
        "selsum": selsum,
    }


def make_in_maps(x, noise, wg_param, wnoise_param):
    consts = _consts()
    wgp = np.repeat(np.ascontiguousarray(wg_param, dtype=np.float32).reshape(C), 8
                    ).reshape(128, 1)
    wnp = np.repeat(np.ascontiguousarray(wnoise_param, dtype=np.float32).reshape(C), 8
                    ).reshape(128, 1)
    x = np.ascontiguousarray(x, dtype=np.float32)
    noise = np.ascontiguousarray(noise, dtype=np.float32)
    in_maps = []
    for i in range(NCORES):
        xs = x[i * BPC:(i + 1) * BPC].reshape(BPC, 128, FB)
        ns = noise[i * BPC:(i + 1) * BPC].reshape(BPC, 128, FB)
        in_maps.append({"x": xs, "noise": ns, "wgp": wgp, "wnp": wnp, **consts})
    return in_maps


def kernel(x, noise, wg_param, wnoise_param):
    from concourse.bass_utils import run_bass_kernel_spmd

    if "nc" not in _CACHE:
        _CACHE["nc"] = _build()
    nc = _CACHE["nc"]
    in_maps = make_in_maps(x, noise, wg_param, wnoise_param)
    res = run_bass_kernel_spmd(nc, in_maps, list(range(NCORES)))
    G = np.empty((B, C, H, W), dtype=np.float32)
    L = np.empty((B, C, H, W), dtype=np.float32)
    for i in range(NCORES):
        G[i * BPC:(i + 1) * BPC] = res.results[i]["g_out"].reshape(BPC, C, H, W)
        L[i * BPC:(i + 1) * BPC] = res.results[i]["load_out"].reshape(BPC, C, H, W)
    return G, L
